# revision 16
# baseline (speedup 1.0000x reference)
"""Trainium2 Bass kernel for the AdsorptionGNN message-passing network.

Strategy (8 NeuronCores, SPMD):
  - Permute nodes into 8*49 blocks of <=128 nodes with <=2048 in-edges each
    (greedy bin packing).  Core k owns 49 blocks (6250 nodes); edges are
    sharded by dst block, so the scatter-add is block-local in PSUM.
  - Per layer, each core computes u = x @ W1a and v = x @ W1b for its own
    nodes; u is AllGather-ed so every core can gather u[src] per edge
    (the only per-edge memory traffic).  The dst contribution uses the
    block-local one-hot matrix P (built on-chip with is_equal) so no
    x[dst] gather is needed:  h^T = u[src]^T + v_b^T @ P^T + W1c^T @ ea^T.
  - msg = silu(h @ ew2 + b2) ;  agg = P^T-matmul accumulation in PSUM;
    x' = silu([x|agg] @ nw + nb)  computed block by block.
  - Graph pooling: per-core partial sums via one-hot batch matmul,
    AllReduce over the 8 cores, then the tiny readout MLP on every core.
"""

import numpy as np

import concourse.bass as bass
import concourse.tile as tile
from concourse import mybir
from concourse.bass_utils import run_bass_kernel_spmd
from concourse.masks import make_identity

# -------------------- problem constants (hardcoded) --------------------
N_NODES = 50000
N_EDGES = 800000
HID = 128
ED = 4
LAYERS = 4
G = 512
NCORES = 8

BPC = 49              # blocks per core
CAP = 2048            # edge capacity per block
SCW = 512             # superchunk width (edges)
SC_PER_B = CAP // SCW # 4
CH_PER_SC = SCW // 128
NCH = BPC * CAP // 128          # 784 chunks of 128 edges per core
NSC = BPC * SC_PER_B            # 196 superchunks per core
SLOTS = BPC * 128               # 6272 node slots per core
EPC = BPC * CAP                 # 100352 padded edges per core

F32 = mybir.dt.float32
I32 = mybir.dt.int32

# misc row offsets (single-partition constants)
IOTA512_OFF = 0
INVC_OFF = 512
RB2_OFF = 1024
MISC_W = 1025


def _pack_nodes(deg):
    """Greedy bin packing: 392 bins, <=128 nodes and <=2048 edges per bin,
    exactly 6250 nodes per core.  Returns (bin_id[n], pos_in_bin[n])."""
    import heapq

    NB = NCORES * BPC
    order = np.argsort(-deg, kind="stable")
    bin_edges = np.zeros(NB, np.int64)
    bin_nodes = np.zeros(NB, np.int64)
    core_nodes = np.zeros(NCORES, np.int64)
    bin_id = np.full(N_NODES, -1, np.int64)
    pos = np.full(N_NODES, -1, np.int64)
    heap = [(-CAP, b) for b in range(NB)]
    heapq.heapify(heap)
    for n in order:
        dn = int(deg[n])
        tmp = []
        placed = False
        while heap:
            negrem, b = heapq.heappop(heap)
            core = b // BPC
            if (
                bin_edges[b] + dn <= CAP
                and bin_nodes[b] < 128
                and core_nodes[core] < N_NODES // NCORES
            ):
                bin_id[n] = b
                pos[n] = bin_nodes[b]
                bin_edges[b] += dn
                bin_nodes[b] += 1
                core_nodes[core] += 1
                heapq.heappush(heap, (-(CAP - bin_edges[b]), b))
                placed = True
                break
            tmp.append((negrem, b))
        for t in tmp:
            heapq.heappush(heap, t)
        if not placed:
            raise RuntimeError("node bin packing failed")
    return bin_id, pos


def _preprocess(z, edge_index, edge_attr, batch):
    """Host-side sharding.  Returns a list of per-core input dicts plus the
    shared-weight entries to be merged in."""
    z = np.asarray(z).astype(np.int64)
    src = np.asarray(edge_index[0]).astype(np.int64)
    dst = np.asarray(edge_index[1]).astype(np.int64)
    ea = np.asarray(edge_attr).astype(np.float32)
    batch = np.asarray(batch).astype(np.int64)

    deg = np.bincount(dst, minlength=N_NODES)
    bin_id, pos = _pack_nodes(deg)

    # global u-table row for each node: core*SLOTS + block*128 + pos
    core_of = bin_id // BPC
    block_of = bin_id % BPC
    urow = core_of * SLOTS + block_of * 128 + pos

    # order edges by dst bin (stable)
    e_bin = bin_id[dst]
    e_order = np.argsort(e_bin, kind="stable")
    sorted_bin = e_bin[e_order]
    # rank of each edge within its bin
    bin_counts = np.bincount(sorted_bin, minlength=NCORES * BPC)
    assert bin_counts.max() <= CAP
    starts = np.zeros(NCORES * BPC + 1, np.int64)
    np.cumsum(bin_counts, out=starts[1:])
    q = np.arange(N_EDGES) - starts[sorted_bin]
    e_core = sorted_bin // BPC
    e_block = sorted_bin % BPC
    j = e_block * CAP + q  # slot within core's padded edge list

    per_core = []
    for c in range(NCORES):
        m = e_core == c
        js = j[m]
        es = e_order[m]
        src_cols = np.zeros((128, NCH), np.int32)
        dst_cols = np.full((128, NCH), -1.0, np.float32)
        ea_pack = np.zeros((4, EPC), np.float32)
        pcol = (js % 128).astype(np.int64)
        ccol = (js // 128).astype(np.int64)
        src_cols[pcol, ccol] = urow[src[es]].astype(np.int32)
        dst_cols[pcol, ccol] = pos[dst[es]].astype(np.float32)
        for d in range(ED):
            ea_pack[d, js] = ea[es, d]

        z_cols = np.zeros((128, BPC), np.int32)
        bv_cols = np.full((128, BPC), -1.0, np.float32)
        nm = core_of == c
        nidx = np.nonzero(nm)[0]
        z_cols[pos[nidx], block_of[nidx]] = z[nidx].astype(np.int32)
        bv_cols[pos[nidx], block_of[nidx]] = batch[nidx].astype(np.float32)
        per_core.append(
            {
                "src_cols": src_cols,
                "dst_cols": dst_cols,
                "ea_pack": ea_pack,
                "z_cols": z_cols,
                "bv_cols": bv_cols,
            }
        )
    return per_core


def _prep_weights(emb, ew1, eb1, ew2, eb2, nw, nb, rw1, rb1, rw2, rb2, batch):
    emb = np.asarray(emb, np.float32)
    ew1 = np.asarray(ew1, np.float32)
    ew2 = np.asarray(ew2, np.float32)
    nw = np.asarray(nw, np.float32)
    rw1 = np.asarray(rw1, np.float32)
    rw2 = np.asarray(rw2, np.float32)

    wpack = np.zeros((128, LAYERS * 640 + 129), np.float32)
    for l in range(LAYERS):
        b = l * 640
        wpack[:, b : b + 128] = ew1[l][:128]
        wpack[:, b + 128 : b + 256] = ew1[l][128:256]
        wpack[:, b + 256 : b + 384] = ew2[l]
        wpack[:, b + 384 : b + 512] = nw[l][:128]
        wpack[:, b + 512 : b + 640] = nw[l][128:]
    wpack[:, LAYERS * 640 : LAYERS * 640 + 128] = rw1
    wpack[:, LAYERS * 640 + 128 : LAYERS * 640 + 129] = rw2

    w1c = np.zeros((4, 128 * LAYERS), np.float32)
    for l in range(LAYERS):
        w1c[:, 128 * l : 128 * (l + 1)] = ew1[l][256:260]

    bpack = np.zeros((128, 3 * LAYERS + 1), np.float32)
    for l in range(LAYERS):
        bpack[:, 3 * l] = np.asarray(eb1[l], np.float32)
        bpack[:, 3 * l + 1] = np.asarray(eb2[l], np.float32)
        bpack[:, 3 * l + 2] = np.asarray(nb[l], np.float32)
    bpack[:, 3 * LAYERS] = np.asarray(rb1, np.float32)

    counts = np.bincount(np.asarray(batch, np.int64), minlength=G).astype(np.float32)
    misc = np.zeros((1, MISC_W), np.float32)
    misc[0, IOTA512_OFF : IOTA512_OFF + 512] = np.arange(512, dtype=np.float32)
    misc[0, INVC_OFF : INVC_OFF + 512] = 1.0 / np.maximum(counts, 1.0)
    misc[0, RB2_OFF] = float(np.asarray(rb2).reshape(-1)[0])

    iota_mat = np.tile(np.arange(128, dtype=np.float32)[None, :], (128, 1))
    iota512_mat = np.tile(np.arange(512, dtype=np.float32)[None, :], (128, 1))
    invc_mat = np.tile(misc[0:1, INVC_OFF : INVC_OFF + 512], (128, 1))

    return {
        "wpack": wpack,
        "w1c": w1c,
        "bpack": bpack,
        "misc": misc,
        "iota_mat": iota_mat,
        "iota512_mat": iota512_mat,
        "invc_mat": invc_mat,
        "emb_t": emb,
    }


def _split_excess_waits(nc, max_waits=1):
    """This walrus build rejects instructions with >1 embedded sync wait.
    Hoist excess waits onto same-engine NoOps inserted just before."""
    cnt = 0
    for f in nc.m.functions:
        for bb in f.blocks:
            new_list = []
            for ins in bb.instructions:
                si = ins.sync_info
                waits = list(si.on_wait) if si and si.on_wait else []
                if len(waits) > max_waits:
                    extra = waits[:-max_waits]
                    si.on_wait = waits[-max_waits:]
                    while extra:
                        chunk, extra = extra[:max_waits], extra[max_waits:]
                        cnt += 1
                        nop = mybir.InstNoOp(
                            name=f"I-waitsplit-{cnt}", engine=ins.engine, ins=[], outs=[]
                        )
                        nop.sync_info = mybir.SyncInfo(on_wait=chunk, on_update=[])
                        new_list.append(nop)
                new_list.append(ins)
            bb.instructions[:] = new_list
    return cnt


def _build_program(n_layers=LAYERS, debug=False, sim_mode=False):
    nc = bass.Bass("TRN2", target_bir_lowering=False)

    ei = lambda name, shape, dt=F32: nc.dram_tensor(name, shape, dt, kind="ExternalInput")
    src_t = ei("src_cols", [128, NCH], I32)
    dstc_t = ei("dst_cols", [128, NCH])
    ea_t = ei("ea_pack", [4, EPC])
    z_t = ei("z_cols", [128, BPC], I32)
    bv_t = ei("bv_cols", [128, BPC])
    w_t = ei("wpack", [128, LAYERS * 640 + 129])
    w1c_t = ei("w1c", [4, 128 * LAYERS])
    b_t = ei("bpack", [128, 3 * LAYERS + 1])
    misc_t = ei("misc", [1, MISC_W])
    iota_t = ei("iota_mat", [128, 128])
    iota512_t = ei("iota512_mat", [128, 512])
    invc_t = ei("invc_mat", [128, 512])
    emb_t = ei("emb_t", [101, 128])
    pred_t = nc.dram_tensor("pred", [1, G], F32, kind="ExternalOutput")
    if debug:
        dbg_x0 = nc.dram_tensor("dbg_x0", [128, SLOTS], F32, kind="ExternalOutput")
        dbg_u = nc.dram_tensor("dbg_u", [SLOTS, 128], F32, kind="ExternalOutput")
        dbg_h = nc.dram_tensor("dbg_h", [128, 512], F32, kind="ExternalOutput")
        dbg_mN = nc.dram_tensor("dbg_mN", [128, 512], F32, kind="ExternalOutput")
        dbg_agg = nc.dram_tensor("dbg_agg", [128, 128], F32, kind="ExternalOutput")
        dbg_x1 = nc.dram_tensor("dbg_x1", [128, SLOTS], F32, kind="ExternalOutput")
        dbg_gg = nc.dram_tensor("dbg_gg", [128, 512], F32, kind="ExternalOutput")
        dbg_us = nc.dram_tensor("dbg_us", [128, 512], F32, kind="ExternalOutput")

    with tile.TileContext(nc) as tc:
        with tc.tile_pool(name="persist", bufs=1) as pp, \
             tc.tile_pool(name="work", bufs=3) as wp, \
             tc.tile_pool(name="gat", bufs=10) as gp, \
             tc.tile_pool(name="ptp", bufs=9) as ptp, \
             tc.tile_pool(name="ps_big", bufs=2, space="PSUM") as pbig, \
             tc.tile_pool(name="ps_mid", bufs=2, space="PSUM") as pmid, \
             tc.tile_pool(name="ps_sm", bufs=2, space="PSUM") as psm, \
             tc.tile_pool(name="ps_agg", bufs=2, space="PSUM") as pagg, \
             tc.tile_pool(name="dram", bufs=1, space="DRAM") as dp:

            # ---------------- persistent SBUF state ----------------
            src_sb = pp.tile([128, NCH], I32)
            dstc_sb = pp.tile([128, NCH], F32)
            z_sb = pp.tile([128, BPC], I32)
            bv_sb = pp.tile([128, BPC], F32)
            w_sb = pp.tile([128, LAYERS * 640 + 129], F32)
            w1c_sb = pp.tile([4, 128 * LAYERS], F32)
            b_sb = pp.tile([128, 3 * LAYERS + 1], F32)
            misc_sb = pp.tile([1, MISC_W], F32)
            iota_sb = pp.tile([128, 128], F32)
            iota512_sb = pp.tile([128, 512], F32)
            invc_sb = pp.tile([128, 512], F32)
            ident_sb = pp.tile([128, 128], F32)
            xA = pp.tile([128, SLOTS], F32)
            xB = pp.tile([128, SLOTS], F32)
            v_sb = pp.tile([128, SLOTS], F32)

            for t, d in [
                (src_sb, src_t), (dstc_sb, dstc_t), (z_sb, z_t),
                (bv_sb, bv_t), (w_sb, w_t), (w1c_sb, w1c_t), (b_sb, b_t),
                (misc_sb, misc_t), (iota_sb, iota_t), (iota512_sb, iota512_t),
                (invc_sb, invc_t),
            ]:
                nc.sync.dma_start(t[:], d[:])
            make_identity(nc, ident_sb[:])

            u_own = dp.tile([SLOTS, 128], F32)
            u_fulls = [
                dp.tile([NCORES * SLOTS, 128], F32, addr_space="Shared",
                        name=f"u_full_l{i}", tag=f"u_full_l{i}")
                for i in range(n_layers)
            ]
            cc_in = dp.tile([128, 512], F32)
            cc_out = dp.tile([128, 512], F32, addr_space="Shared")

            WL = lambda l, k: w_sb[:, l * 640 + k * 128 : l * 640 + (k + 1) * 128]

            # ---------------- x0 = emb[z] (feature-major in SBUF) ----------------
            for b in range(BPC):
                gz = gp.tile([128, 128], F32, tag="gz")
                nc.gpsimd.indirect_dma_start(
                    out=gz[:], out_offset=None, in_=emb_t[:],
                    in_offset=bass.IndirectOffsetOnAxis(ap=z_sb[:, b : b + 1], axis=0),
                )
                ps = psm.tile([128, 128], F32, tag="sm")
                nc.tensor.matmul(ps[:], lhsT=gz[:], rhs=ident_sb[:],
                                 is_transpose=True, start=True, stop=True)
                nc.vector.tensor_copy(xA[:, b * 128 : (b + 1) * 128], ps[:])
            if debug:
                nc.sync.dma_start(dbg_x0[:], xA[:])

            # ---------------- layers ----------------
            for l in range(n_layers):
                x_cur = xA if l % 2 == 0 else xB
                x_nxt = xB if l % 2 == 0 else xA

                # --- phase 1: u/v for own nodes; write u to DRAM ---
                for b in range(BPC):
                    bs = b * 128
                    ps_u = psm.tile([128, 128], F32, tag="sm")
                    ps_v = psm.tile([128, 128], F32, tag="sm")
                    nc.tensor.matmul(ps_u[:], lhsT=x_cur[:, bs : bs + 128],
                                     rhs=WL(l, 0), start=True, stop=True)
                    nc.tensor.matmul(ps_v[:], lhsT=x_cur[:, bs : bs + 128],
                                     rhs=WL(l, 1), start=True, stop=True)
                    ust = wp.tile([128, 128], F32, tag="ust")
                    nc.scalar.copy(ust[:], ps_u[:])
                    nc.vector.tensor_copy(v_sb[:, bs : bs + 128], ps_v[:])
                    nc.sync.dma_start(u_own[bs : bs + 128, :], ust[:])
                    if debug and l == 0:
                        nc.sync.dma_start(dbg_u[bs : bs + 128, :], ust[:])

                # --- AllGather u across the 8 cores ---
                u_full = u_fulls[l]
                if sim_mode:
                    nc.sync.dma_start(u_full.opt()[0:SLOTS, :], u_own.opt()[:, :])
                else:
                    nc.gpsimd.collective_compute(
                        "AllGather", mybir.AluOpType.bypass,
                        replica_groups=[list(range(NCORES))],
                        ins=[u_own.opt()], outs=[u_full.opt()],
                    )

                # --- phase 2: edges ---
                for b in range(BPC):
                    bs = b * 128
                    ps_agg = pagg.tile([128, 128], F32, tag="agg")
                    for sc in range(SC_PER_B):
                        s = b * SC_PER_B + sc
                        ps_h = pbig.tile([128, 512], F32, tag="ps_h")
                        ptiles = []
                        gtiles = []
                        for ccn in range(CH_PER_SC):
                            c = s * CH_PER_SC + ccn
                            g = gp.tile([128, 128], F32, tag="g")
                            nc.gpsimd.indirect_dma_start(
                                out=g[:], out_offset=None, in_=u_full.opt(),
                                in_offset=bass.IndirectOffsetOnAxis(
                                    ap=src_sb[:, c : c + 1], axis=0),
                            )
                            gtiles.append(g)
                            # one-hot P for this chunk (edges x nodes)
                            pt = ptp.tile([128, 128], F32, tag="pt")
                            nc.vector.tensor_tensor(
                                out=pt[:],
                                in0=dstc_sb[:, c : c + 1].to_broadcast([128, 128]),
                                in1=iota_sb[:],
                                op=mybir.AluOpType.is_equal)
                            ptiles.append(pt)
                        # P^T assembled via PE transpose
                        pT = wp.tile([128, 512], F32, tag="pT")
                        for ccn in range(CH_PER_SC):
                            ps_pt = psm.tile([128, 128], F32, tag="sm")
                            nc.tensor.matmul(ps_pt[:], lhsT=ptiles[ccn][:],
                                             rhs=ident_sb[:], is_transpose=True,
                                             start=True, stop=True)
                            nc.vector.tensor_copy(
                                pT[:, ccn * 128 : (ccn + 1) * 128], ps_pt[:])
                        # h^T += v_b^T P^T  +  W1c^T ea^T
                        ea_sl = wp.tile([4, 512], F32, tag="ea_sl")
                        nc.sync.dma_start(ea_sl[:], ea_t[:, s * SCW : s * SCW + SCW])
                        nc.tensor.matmul(ps_h[:], lhsT=v_sb[:, bs : bs + 128],
                                         rhs=pT[:], start=True, stop=False)
                        nc.tensor.matmul(
                            ps_h[:], lhsT=w1c_sb[:, 128 * l : 128 * (l + 1)],
                            rhs=ea_sl[:], start=False, stop=False)
                        for ccn in range(CH_PER_SC):
                            nc.tensor.matmul(
                                ps_h[:, ccn * 128 : (ccn + 1) * 128],
                                lhsT=gtiles[ccn][:],
                                rhs=ident_sb[:], is_transpose=True,
                                start=False, stop=(ccn == CH_PER_SC - 1))
                        hT = wp.tile([128, 512], F32, tag="hT")
                        nc.scalar.activation(hT[:], ps_h[:],
                                             mybir.ActivationFunctionType.Silu,
                                             bias=b_sb[:, 3 * l : 3 * l + 1])
                        if debug and l == 0 and s == 0:
                            nc.sync.dma_start(dbg_h[:], hT[:])
                            dbg_us_t = wp.tile([128, 512], F32, tag="dbg_us_t")
                            nc.vector.tensor_copy(dbg_us_t[:], ps_h[:])
                            nc.sync.dma_start(dbg_us[:], dbg_us_t[:])
                        ps_m = pmid.tile([128, 512], F32, tag="ps_mn")
                        nc.tensor.matmul(ps_m[:], lhsT=WL(l, 2), rhs=hT[:],
                                         start=True, stop=True)
                        mT = wp.tile([128, 512], F32, tag="mT")
                        nc.scalar.activation(mT[:], ps_m[:],
                                             mybir.ActivationFunctionType.Silu,
                                             bias=b_sb[:, 3 * l + 1 : 3 * l + 2])
                        # transpose msg to edge-major, then scatter into agg
                        ps_n = pmid.tile([128, 512], F32, tag="ps_mn")
                        for ccn in range(CH_PER_SC):
                            nc.tensor.matmul(
                                ps_n[:, ccn * 128 : (ccn + 1) * 128],
                                lhsT=mT[:, ccn * 128 : (ccn + 1) * 128],
                                rhs=ident_sb[:], is_transpose=True,
                                start=True, stop=True)
                        mN = wp.tile([128, 512], F32, tag="mN")
                        nc.vector.tensor_copy(mN[:], ps_n[:])
                        if debug and l == 0 and s == 0:
                            nc.sync.dma_start(dbg_mN[:], mN[:])
                        for ccn in range(CH_PER_SC):
                            nc.tensor.matmul(
                                ps_agg[:], lhsT=ptiles[ccn][:],
                                rhs=mN[:, ccn * 128 : (ccn + 1) * 128],
                                start=(sc == 0 and ccn == 0),
                                stop=(sc == SC_PER_B - 1 and ccn == CH_PER_SC - 1))
                    # --- node update for block b ---
                    aggS = wp.tile([128, 128], F32, tag="aggS")
                    nc.vector.tensor_copy(aggS[:], ps_agg[:])
                    if debug and l == 0 and b == 0:
                        nc.sync.dma_start(dbg_agg[:], aggS[:])
                    ps_aT = psm.tile([128, 128], F32, tag="sm")
                    nc.tensor.matmul(ps_aT[:], lhsT=aggS[:], rhs=ident_sb[:],
                                     is_transpose=True, start=True, stop=True)
                    aggT = wp.tile([128, 128], F32, tag="aggT")
                    nc.vector.tensor_copy(aggT[:], ps_aT[:])
                    ps_x = psm.tile([128, 128], F32, tag="sm")
                    nc.tensor.matmul(ps_x[:], lhsT=WL(l, 3),
                                     rhs=x_cur[:, bs : bs + 128],
                                     start=True, stop=False)
                    nc.tensor.matmul(ps_x[:], lhsT=WL(l, 4), rhs=aggT[:],
                                     start=False, stop=True)
                    nc.scalar.activation(x_nxt[:, bs : bs + 128], ps_x[:],
                                         mybir.ActivationFunctionType.Silu,
                                         bias=b_sb[:, 3 * l + 2 : 3 * l + 3])

            # ---------------- graph pooling + readout ----------------
            x_fin = xA if n_layers % 2 == 0 else xB
            if debug:
                nc.sync.dma_start(dbg_x1[:], x_fin[:])
            ps_gg = pagg.tile([128, 512], F32, tag="agg")
            for b in range(BPC):
                bs = b * 128
                ps_xn = psm.tile([128, 128], F32, tag="sm")
                nc.tensor.matmul(ps_xn[:], lhsT=x_fin[:, bs : bs + 128],
                                 rhs=ident_sb[:], is_transpose=True,
                                 start=True, stop=True)
                xn = wp.tile([128, 128], F32, tag="xn")
                nc.vector.tensor_copy(xn[:], ps_xn[:])
                Bt = wp.tile([128, 512], F32, tag="Bt")
                nc.vector.tensor_tensor(
                    out=Bt[:],
                    in0=bv_sb[:, b : b + 1].to_broadcast([128, 512]),
                    in1=iota512_sb[:],
                    op=mybir.AluOpType.is_equal)
                nc.tensor.matmul(ps_gg[:], lhsT=xn[:], rhs=Bt[:],
                                 start=(b == 0), stop=(b == BPC - 1))
            ggS = wp.tile([128, 512], F32, tag="ggS")
            nc.vector.tensor_copy(ggS[:], ps_gg[:])
            if debug:
                nc.sync.dma_start(dbg_gg[:], ggS[:])
            nc.gpsimd.dma_start(cc_in[:], ggS[:])
            if sim_mode:
                nc.sync.dma_start(cc_out.opt()[:, :], cc_in.opt()[:, :])
            else:
                nc.gpsimd.collective_compute(
                    "AllReduce", mybir.AluOpType.add,
                    replica_groups=[list(range(NCORES))],
                    ins=[cc_in.opt()], outs=[cc_out.opt()],
                )
            gsum = wp.tile([128, 512], F32, tag="gsum")
            nc.gpsimd.dma_start(gsum[:], cc_out[:])
            gmean = wp.tile([128, 512], F32, tag="gmean")
            nc.vector.tensor_tensor(out=gmean[:], in0=gsum[:], in1=invc_sb[:],
                                    op=mybir.AluOpType.mult)
            ps_y = pmid.tile([128, 512], F32, tag="ps_mn")
            nc.tensor.matmul(ps_y[:],
                             lhsT=w_sb[:, LAYERS * 640 : LAYERS * 640 + 128],
                             rhs=gmean[:], start=True, stop=True)
            y1 = wp.tile([128, 512], F32, tag="y1")
            nc.scalar.activation(y1[:], ps_y[:],
                                 mybir.ActivationFunctionType.Silu,
                                 bias=b_sb[:, 3 * LAYERS : 3 * LAYERS + 1])
            ps_p = psm.tile([1, 512], F32, tag="sm")
            nc.tensor.matmul(
                ps_p[:],
                lhsT=w_sb[:, LAYERS * 640 + 128 : LAYERS * 640 + 129],
                rhs=y1[:], start=True, stop=True)
            predS = wp.tile([1, 512], F32, tag="predS")
            nc.vector.tensor_tensor(
                out=predS[:], in0=ps_p[:],
                in1=misc_sb[0:1, RB2_OFF : RB2_OFF + 1].to_broadcast([1, 512]),
                op=mybir.AluOpType.add)
            nc.sync.dma_start(pred_t[:], predS[:])

    _split_excess_waits(nc)
    return nc


_prog_cache = {}


def kernel(**inputs) -> np.ndarray:
    per_core = _preprocess(
        inputs["z"], inputs["edge_index"], inputs["edge_attr"], inputs["batch"]
    )
    shared = _prep_weights(
        inputs["emb"], inputs["ew1"], inputs["eb1"], inputs["ew2"], inputs["eb2"],
        inputs["nw"], inputs["nb"], inputs["rw1"], inputs["rb1"], inputs["rw2"],
        inputs["rb2"], inputs["batch"],
    )
    if "prog" not in _prog_cache:
        _prog_cache["prog"] = _build_program()
    nc = _prog_cache["prog"]
    in_maps = [{**pc, **shared} for pc in per_core]
    res = run_bass_kernel_spmd(nc, in_maps, core_ids=list(range(NCORES)))
    return np.asarray(res.results[0]["pred"]).reshape(G).astype(np.float32)


# revision 24
# speedup vs baseline: 1.4443x; 1.4443x over previous
"""Trainium2 Bass kernel for the AdsorptionGNN message-passing network.

Strategy (8 NeuronCores, SPMD):
  - Permute nodes into 8*49 blocks of <=128 nodes with <=2048 in-edges each
    (greedy bin packing).  Core k owns 49 blocks (6250 nodes); edges are
    sharded by dst block, so the scatter-add is block-local in PSUM.
  - Per layer, each core computes u = x @ W1a and v = x @ W1b for its own
    nodes; u is AllGather-ed so every core can gather u[src] per edge
    (the only per-edge memory traffic).  The dst contribution uses the
    block-local one-hot matrix P (built on-chip with is_equal) so no
    x[dst] gather is needed:  h^T = u[src]^T + v_b^T @ P^T + W1c^T @ ea^T.
  - msg = silu(h @ ew2 + b2) ;  agg = P^T-matmul accumulation in PSUM;
    x' = silu([x|agg] @ nw + nb)  computed block by block.
  - Graph pooling: per-core partial sums via one-hot batch matmul,
    AllReduce over the 8 cores, then the tiny readout MLP on every core.
"""

import numpy as np

import concourse.bass as bass
import concourse.tile as tile
from concourse import mybir
from concourse.bass_utils import run_bass_kernel_spmd
from concourse.masks import make_identity

# -------------------- problem constants (hardcoded) --------------------
N_NODES = 50000
N_EDGES = 800000
HID = 128
ED = 4
LAYERS = 4
G = 512
NCORES = 8

BPC = 49              # blocks per core
CAP = 2048            # edge capacity per block
SCW = 512             # superchunk width (edges)
SC_PER_B = CAP // SCW # 4
CH_PER_SC = SCW // 128
NCH = BPC * CAP // 128          # 784 chunks of 128 edges per core
NSC = BPC * SC_PER_B            # 196 superchunks per core
SLOTS = BPC * 128               # 6272 node slots per core
EPC = BPC * CAP                 # 100352 padded edges per core

F32 = mybir.dt.float32
I32 = mybir.dt.int32

# misc row offsets (single-partition constants)
IOTA512_OFF = 0
INVC_OFF = 512
RB2_OFF = 1024
MISC_W = 1025


def _pack_nodes(deg):
    """Greedy bin packing: 392 bins, <=128 nodes and <=2048 edges per bin,
    exactly 6250 nodes per core.  Returns (bin_id[n], pos_in_bin[n])."""
    import heapq

    NB = NCORES * BPC
    order = np.argsort(-deg, kind="stable")
    bin_edges = np.zeros(NB, np.int64)
    bin_nodes = np.zeros(NB, np.int64)
    core_nodes = np.zeros(NCORES, np.int64)
    bin_id = np.full(N_NODES, -1, np.int64)
    pos = np.full(N_NODES, -1, np.int64)
    heap = [(-CAP, b) for b in range(NB)]
    heapq.heapify(heap)
    for n in order:
        dn = int(deg[n])
        tmp = []
        placed = False
        while heap:
            negrem, b = heapq.heappop(heap)
            core = b // BPC
            if (
                bin_edges[b] + dn <= CAP
                and bin_nodes[b] < 128
                and core_nodes[core] < N_NODES // NCORES
            ):
                bin_id[n] = b
                pos[n] = bin_nodes[b]
                bin_edges[b] += dn
                bin_nodes[b] += 1
                core_nodes[core] += 1
                heapq.heappush(heap, (-(CAP - bin_edges[b]), b))
                placed = True
                break
            tmp.append((negrem, b))
        for t in tmp:
            heapq.heappush(heap, t)
        if not placed:
            raise RuntimeError("node bin packing failed")
    return bin_id, pos


def _preprocess(z, edge_index, edge_attr, batch):
    """Host-side sharding.  Returns a list of per-core input dicts plus the
    shared-weight entries to be merged in."""
    z = np.asarray(z).astype(np.int64)
    src = np.asarray(edge_index[0]).astype(np.int64)
    dst = np.asarray(edge_index[1]).astype(np.int64)
    ea = np.asarray(edge_attr).astype(np.float32)
    batch = np.asarray(batch).astype(np.int64)

    deg = np.bincount(dst, minlength=N_NODES)
    bin_id, pos = _pack_nodes(deg)

    # global u-table row for each node: core*SLOTS + block*128 + pos
    core_of = bin_id // BPC
    block_of = bin_id % BPC
    urow = core_of * SLOTS + block_of * 128 + pos

    # order edges by dst bin (stable)
    e_bin = bin_id[dst]
    e_order = np.argsort(e_bin, kind="stable")
    sorted_bin = e_bin[e_order]
    # rank of each edge within its bin
    bin_counts = np.bincount(sorted_bin, minlength=NCORES * BPC)
    assert bin_counts.max() <= CAP
    starts = np.zeros(NCORES * BPC + 1, np.int64)
    np.cumsum(bin_counts, out=starts[1:])
    q = np.arange(N_EDGES) - starts[sorted_bin]
    e_core = sorted_bin // BPC
    e_block = sorted_bin % BPC
    j = e_block * CAP + q  # slot within core's padded edge list

    per_core = []
    for c in range(NCORES):
        m = e_core == c
        js = j[m]
        es = e_order[m]
        src_cols = np.zeros((128, NCH), np.int32)
        dst_cols = np.full((128, NCH), -1.0, np.float32)
        ea_pack = np.zeros((4, EPC), np.float32)
        pcol = (js % 128).astype(np.int64)
        ccol = (js // 128).astype(np.int64)
        src_cols[pcol, ccol] = urow[src[es]].astype(np.int32)
        dst_cols[pcol, ccol] = pos[dst[es]].astype(np.float32)
        for d in range(ED):
            ea_pack[d, js] = ea[es, d]
        dstr = np.full((1, EPC), -1.0, np.float32)
        dstr[0, js] = pos[dst[es]].astype(np.float32)

        z_cols = np.zeros((128, BPC), np.int32)
        bv_cols = np.full((128, BPC), -1.0, np.float32)
        nm = core_of == c
        nidx = np.nonzero(nm)[0]
        z_cols[pos[nidx], block_of[nidx]] = z[nidx].astype(np.int32)
        bv_cols[pos[nidx], block_of[nidx]] = batch[nidx].astype(np.float32)
        per_core.append(
            {
                "src_cols": src_cols,
                "dst_cols": dst_cols,
                "ea_pack": ea_pack,
                "dstr": dstr,
                "z_cols": z_cols,
                "bv_cols": bv_cols,
            }
        )
    return per_core


def _prep_weights(emb, ew1, eb1, ew2, eb2, nw, nb, rw1, rb1, rw2, rb2, batch):
    emb = np.asarray(emb, np.float32)
    ew1 = np.asarray(ew1, np.float32)
    ew2 = np.asarray(ew2, np.float32)
    nw = np.asarray(nw, np.float32)
    rw1 = np.asarray(rw1, np.float32)
    rw2 = np.asarray(rw2, np.float32)

    wpack = np.zeros((128, LAYERS * 640 + 129), np.float32)
    for l in range(LAYERS):
        b = l * 640
        wpack[:, b : b + 128] = ew1[l][:128]
        wpack[:, b + 128 : b + 256] = ew1[l][128:256]
        wpack[:, b + 256 : b + 384] = ew2[l]
        wpack[:, b + 384 : b + 512] = nw[l][:128]
        wpack[:, b + 512 : b + 640] = nw[l][128:]
    wpack[:, LAYERS * 640 : LAYERS * 640 + 128] = rw1
    wpack[:, LAYERS * 640 + 128 : LAYERS * 640 + 129] = rw2

    w1c = np.zeros((4, 128 * LAYERS), np.float32)
    for l in range(LAYERS):
        w1c[:, 128 * l : 128 * (l + 1)] = ew1[l][256:260]

    bpack = np.zeros((128, 3 * LAYERS + 1), np.float32)
    for l in range(LAYERS):
        bpack[:, 3 * l] = np.asarray(eb1[l], np.float32)
        bpack[:, 3 * l + 1] = np.asarray(eb2[l], np.float32)
        bpack[:, 3 * l + 2] = np.asarray(nb[l], np.float32)
    bpack[:, 3 * LAYERS] = np.asarray(rb1, np.float32)

    counts = np.bincount(np.asarray(batch, np.int64), minlength=G).astype(np.float32)
    misc = np.zeros((1, MISC_W), np.float32)
    misc[0, IOTA512_OFF : IOTA512_OFF + 512] = np.arange(512, dtype=np.float32)
    misc[0, INVC_OFF : INVC_OFF + 512] = 1.0 / np.maximum(counts, 1.0)
    misc[0, RB2_OFF] = float(np.asarray(rb2).reshape(-1)[0])

    iota_mat = np.tile(np.arange(128, dtype=np.float32)[None, :], (128, 1))
    iota_col = np.arange(128, dtype=np.float32).reshape(128, 1)
    iota512_mat = np.tile(np.arange(512, dtype=np.float32)[None, :], (128, 1))
    invc_mat = np.tile(misc[0:1, INVC_OFF : INVC_OFF + 512], (128, 1))

    return {
        "wpack": wpack,
        "w1c": w1c,
        "bpack": bpack,
        "misc": misc,
        "iota_mat": iota_mat,
        "iota_col": iota_col,
        "iota512_mat": iota512_mat,
        "invc_mat": invc_mat,
        "emb_t": emb,
    }


def _split_excess_waits(nc, max_waits=1):
    """This walrus build rejects instructions with >1 embedded sync wait.
    Hoist excess waits onto same-engine NoOps inserted just before."""
    cnt = 0
    for f in nc.m.functions:
        for bb in f.blocks:
            new_list = []
            for ins in bb.instructions:
                si = ins.sync_info
                waits = list(si.on_wait) if si and si.on_wait else []
                if len(waits) > max_waits:
                    extra = waits[:-max_waits]
                    si.on_wait = waits[-max_waits:]
                    while extra:
                        chunk, extra = extra[:max_waits], extra[max_waits:]
                        cnt += 1
                        nop = mybir.InstNoOp(
                            name=f"I-waitsplit-{cnt}", engine=ins.engine, ins=[], outs=[]
                        )
                        nop.sync_info = mybir.SyncInfo(on_wait=chunk, on_update=[])
                        new_list.append(nop)
                new_list.append(ins)
            bb.instructions[:] = new_list
    return cnt


def _build_program(n_layers=LAYERS, debug=False, sim_mode=False):
    nc = bass.Bass("TRN2", target_bir_lowering=False)

    ei = lambda name, shape, dt=F32: nc.dram_tensor(name, shape, dt, kind="ExternalInput")
    src_t = ei("src_cols", [128, NCH], I32)
    dstc_t = ei("dst_cols", [128, NCH])
    ea_t = ei("ea_pack", [4, EPC])
    dstr_t = ei("dstr", [1, EPC])
    z_t = ei("z_cols", [128, BPC], I32)
    bv_t = ei("bv_cols", [128, BPC])
    w_t = ei("wpack", [128, LAYERS * 640 + 129])
    w1c_t = ei("w1c", [4, 128 * LAYERS])
    b_t = ei("bpack", [128, 3 * LAYERS + 1])
    misc_t = ei("misc", [1, MISC_W])
    iota_t = ei("iota_mat", [128, 128])
    iotac_t = ei("iota_col", [128, 1])
    iota512_t = ei("iota512_mat", [128, 512])
    invc_t = ei("invc_mat", [128, 512])
    emb_t = ei("emb_t", [101, 128])
    pred_t = nc.dram_tensor("pred", [1, G], F32, kind="ExternalOutput")
    if debug:
        dbg_x0 = nc.dram_tensor("dbg_x0", [128, SLOTS], F32, kind="ExternalOutput")
        dbg_u = nc.dram_tensor("dbg_u", [SLOTS, 128], F32, kind="ExternalOutput")
        dbg_h = nc.dram_tensor("dbg_h", [128, 512], F32, kind="ExternalOutput")
        dbg_mN = nc.dram_tensor("dbg_mN", [128, 512], F32, kind="ExternalOutput")
        dbg_agg = nc.dram_tensor("dbg_agg", [128, 128], F32, kind="ExternalOutput")
        dbg_x1 = nc.dram_tensor("dbg_x1", [128, SLOTS], F32, kind="ExternalOutput")
        dbg_gg = nc.dram_tensor("dbg_gg", [128, 512], F32, kind="ExternalOutput")
        dbg_us = nc.dram_tensor("dbg_us", [128, 512], F32, kind="ExternalOutput")

    with tile.TileContext(nc) as tc:
        with tc.tile_pool(name="persist", bufs=1) as pp, \
             tc.tile_pool(name="work", bufs=3) as wp, \
             tc.tile_pool(name="gat", bufs=(12 if debug else 20)) as gp, \
             tc.tile_pool(name="ptp", bufs=4) as ptp, \
             tc.tile_pool(name="ps_big", bufs=2, space="PSUM") as pbig, \
             tc.tile_pool(name="ps_mid", bufs=2, space="PSUM") as pmid, \
             tc.tile_pool(name="ps_sm", bufs=2, space="PSUM") as psm, \
             tc.tile_pool(name="ps_agg", bufs=2, space="PSUM") as pagg, \
             tc.tile_pool(name="dram", bufs=1, space="DRAM") as dp:

            # ---------------- persistent SBUF state ----------------
            src_sb = pp.tile([128, NCH], I32)
            dstc_sb = pp.tile([128, NCH], F32)
            z_sb = pp.tile([128, BPC], I32)
            bv_sb = pp.tile([128, BPC], F32)
            w_sb = pp.tile([128, LAYERS * 640 + 129], F32)
            w1c_sb = pp.tile([4, 128 * LAYERS], F32)
            b_sb = pp.tile([128, 3 * LAYERS + 1], F32)
            misc_sb = pp.tile([1, MISC_W], F32)
            iota_sb = pp.tile([128, 128], F32)
            iotac_sb = pp.tile([128, 1], F32)
            iota512_sb = pp.tile([128, 512], F32)
            invc_sb = pp.tile([128, 512], F32)
            ident_sb = pp.tile([128, 128], F32)
            xA = pp.tile([128, SLOTS], F32)
            xB = pp.tile([128, SLOTS], F32)
            v_sb = pp.tile([128, SLOTS], F32)

            for t, d in [
                (src_sb, src_t), (dstc_sb, dstc_t), (z_sb, z_t),
                (bv_sb, bv_t), (w_sb, w_t), (w1c_sb, w1c_t), (b_sb, b_t),
                (misc_sb, misc_t), (iota_sb, iota_t), (iotac_sb, iotac_t),
                (iota512_sb, iota512_t),
                (invc_sb, invc_t),
            ]:
                nc.sync.dma_start(t[:], d[:])
            make_identity(nc, ident_sb[:])

            u_own = dp.tile([SLOTS, 128], F32)
            u_fulls = [
                dp.tile([NCORES * SLOTS, 128], F32, addr_space="Shared",
                        name=f"u_full_l{i}", tag=f"u_full_l{i}")
                for i in range(n_layers)
            ]
            cc_in = dp.tile([128, 512], F32)
            cc_out = dp.tile([128, 512], F32, addr_space="Shared")

            WL = lambda l, k: w_sb[:, l * 640 + k * 128 : l * 640 + (k + 1) * 128]

            # ---------------- x0 = emb[z] (feature-major in SBUF) ----------------
            for b in range(BPC):
                gz = gp.tile([128, 128], F32, tag="gz")
                nc.gpsimd.indirect_dma_start(
                    out=gz[:], out_offset=None, in_=emb_t[:],
                    in_offset=bass.IndirectOffsetOnAxis(ap=z_sb[:, b : b + 1], axis=0),
                )
                ps = psm.tile([128, 128], F32, tag="sm")
                nc.tensor.matmul(ps[:], lhsT=gz[:], rhs=ident_sb[:],
                                 is_transpose=True, start=True, stop=True)
                nc.vector.tensor_copy(xA[:, b * 128 : (b + 1) * 128], ps[:])
            if debug:
                nc.sync.dma_start(dbg_x0[:], xA[:])

            # ---------------- layers ----------------
            for l in range(n_layers):
                x_cur = xA if l % 2 == 0 else xB
                x_nxt = xB if l % 2 == 0 else xA

                # --- phase 1: u/v for own nodes; write u to DRAM ---
                for b in range(BPC):
                    bs = b * 128
                    ps_u = psm.tile([128, 128], F32, tag="sm")
                    ps_v = psm.tile([128, 128], F32, tag="sm")
                    nc.tensor.matmul(ps_u[:], lhsT=x_cur[:, bs : bs + 128],
                                     rhs=WL(l, 0), start=True, stop=True)
                    nc.tensor.matmul(ps_v[:], lhsT=x_cur[:, bs : bs + 128],
                                     rhs=WL(l, 1), start=True, stop=True)
                    ust = wp.tile([128, 128], F32, tag="ust")
                    nc.scalar.copy(ust[:], ps_u[:])
                    nc.vector.tensor_copy(v_sb[:, bs : bs + 128], ps_v[:])
                    nc.sync.dma_start(u_own[bs : bs + 128, :], ust[:])
                    if debug and l == 0:
                        nc.sync.dma_start(dbg_u[bs : bs + 128, :], ust[:])

                # --- AllGather u across the 8 cores ---
                u_full = u_fulls[l]
                if sim_mode:
                    nc.sync.dma_start(u_full.opt()[0:SLOTS, :], u_own.opt()[:, :])
                else:
                    nc.gpsimd.collective_compute(
                        "AllGather", mybir.AluOpType.bypass,
                        replica_groups=[list(range(NCORES))],
                        ins=[u_own.opt()], outs=[u_full.opt()],
                    )

                # --- phase 2: edges ---
                for b in range(BPC):
                    bs = b * 128
                    ps_agg = pagg.tile([128, 128], F32, tag="agg")
                    for sc in range(SC_PER_B):
                        s = b * SC_PER_B + sc
                        ps_h = pbig.tile([128, 512], F32, tag="ps_h")
                        ptiles = []
                        gtiles = []
                        for ccn in range(CH_PER_SC):
                            c = s * CH_PER_SC + ccn
                            g = gp.tile([128, 128], F32, tag="g")
                            nc.gpsimd.indirect_dma_start(
                                out=g[:], out_offset=None, in_=u_full.opt(),
                                in_offset=bass.IndirectOffsetOnAxis(
                                    ap=src_sb[:, c : c + 1], axis=0),
                            )
                            gtiles.append(g)
                        # one-hot P for all 4 chunks in one op:
                        # in0[p, ccn*128+n] = dstc[p, c0+ccn]; in1 repeats iota
                        c0 = s * CH_PER_SC
                        pt4 = ptp.tile([128, 512], F32, tag="pt")
                        nc.vector.tensor_tensor(
                            out=pt4[:],
                            in0=dstc_sb[:, c0 : c0 + 4].rearrange(
                                "p (c o) -> p c o", o=1).to_broadcast([128, 4, 128]),
                            in1=iota_sb[:].rearrange(
                                "p (o n) -> p o n", o=1).to_broadcast([128, 4, 128]),
                            op=mybir.AluOpType.is_equal)
                        ptiles = [pt4[:, ccn * 128 : (ccn + 1) * 128]
                                  for ccn in range(CH_PER_SC)]
                        # P^T built directly: broadcast dst row across
                        # partitions during DMA, then one is_equal vs iota col
                        dstr_bc = wp.tile([128, 512], F32, tag="dstr_bc")
                        nc.sync.dma_start(
                            dstr_bc[:],
                            dstr_t[0:1, s * SCW : s * SCW + SCW].to_broadcast(
                                [128, 512]))
                        pT = wp.tile([128, 512], F32, tag="pT")
                        nc.vector.tensor_tensor(
                            out=pT[:],
                            in0=iotac_sb[:, 0:1].to_broadcast([128, 512]),
                            in1=dstr_bc[:],
                            op=mybir.AluOpType.is_equal)
                        # h^T += v_b^T P^T  +  W1c^T ea^T
                        ea_sl = wp.tile([4, 512], F32, tag="ea_sl")
                        nc.sync.dma_start(ea_sl[:], ea_t[:, s * SCW : s * SCW + SCW])
                        nc.tensor.matmul(ps_h[:], lhsT=v_sb[:, bs : bs + 128],
                                         rhs=pT[:], start=True, stop=False)
                        nc.tensor.matmul(
                            ps_h[:], lhsT=w1c_sb[:, 128 * l : 128 * (l + 1)],
                            rhs=ea_sl[:], start=False, stop=False)
                        for ccn in range(CH_PER_SC):
                            nc.tensor.matmul(
                                ps_h[:, ccn * 128 : (ccn + 1) * 128],
                                lhsT=gtiles[ccn][:],
                                rhs=ident_sb[:], is_transpose=True,
                                start=False, stop=(ccn == CH_PER_SC - 1))
                        hT = wp.tile([128, 512], F32, tag="hT")
                        nc.scalar.activation(hT[:], ps_h[:],
                                             mybir.ActivationFunctionType.Silu,
                                             bias=b_sb[:, 3 * l : 3 * l + 1])
                        if debug and l == 0 and s == 0:
                            nc.sync.dma_start(dbg_h[:], hT[:])
                            dbg_us_t = wp.tile([128, 512], F32, tag="dbg_us_t")
                            nc.vector.tensor_copy(dbg_us_t[:], ps_h[:])
                            nc.sync.dma_start(dbg_us[:], dbg_us_t[:])
                        ps_m = pmid.tile([128, 512], F32, tag="ps_mn")
                        nc.tensor.matmul(ps_m[:], lhsT=WL(l, 2), rhs=hT[:],
                                         start=True, stop=True)
                        mT = wp.tile([128, 512], F32, tag="mT")
                        nc.scalar.activation(mT[:], ps_m[:],
                                             mybir.ActivationFunctionType.Silu,
                                             bias=b_sb[:, 3 * l + 1 : 3 * l + 2])
                        # transpose msg to edge-major via PE
                        ps_n = pmid.tile([128, 512], F32, tag="ps_mn")
                        for ccn in range(CH_PER_SC):
                            nc.tensor.matmul(
                                ps_n[:, ccn * 128 : (ccn + 1) * 128],
                                lhsT=mT[:, ccn * 128 : (ccn + 1) * 128],
                                rhs=ident_sb[:], is_transpose=True,
                                start=True, stop=True)
                        mN = wp.tile([128, 512], F32, tag="mN")
                        nc.vector.tensor_copy(mN[:], ps_n[:])
                        if debug and l == 0 and s == 0:
                            nc.sync.dma_start(dbg_mN[:], mN[:])
                        for ccn in range(CH_PER_SC):
                            nc.tensor.matmul(
                                ps_agg[:], lhsT=ptiles[ccn],
                                rhs=mN[:, ccn * 128 : (ccn + 1) * 128],
                                start=(sc == 0 and ccn == 0),
                                stop=(sc == SC_PER_B - 1 and ccn == CH_PER_SC - 1))
                    # --- node update for block b ---
                    aggS = wp.tile([128, 128], F32, tag="aggS")
                    nc.vector.tensor_copy(aggS[:], ps_agg[:])
                    if debug and l == 0 and b == 0:
                        nc.sync.dma_start(dbg_agg[:], aggS[:])
                    ps_aT = psm.tile([128, 128], F32, tag="sm")
                    nc.tensor.matmul(ps_aT[:], lhsT=aggS[:], rhs=ident_sb[:],
                                     is_transpose=True, start=True, stop=True)
                    aggT = wp.tile([128, 128], F32, tag="aggT")
                    nc.vector.tensor_copy(aggT[:], ps_aT[:])
                    ps_x = psm.tile([128, 128], F32, tag="sm")
                    nc.tensor.matmul(ps_x[:], lhsT=WL(l, 3),
                                     rhs=x_cur[:, bs : bs + 128],
                                     start=True, stop=False)
                    nc.tensor.matmul(ps_x[:], lhsT=WL(l, 4), rhs=aggT[:],
                                     start=False, stop=True)
                    nc.scalar.activation(x_nxt[:, bs : bs + 128], ps_x[:],
                                         mybir.ActivationFunctionType.Silu,
                                         bias=b_sb[:, 3 * l + 2 : 3 * l + 3])

            # ---------------- graph pooling + readout ----------------
            x_fin = xA if n_layers % 2 == 0 else xB
            if debug:
                nc.sync.dma_start(dbg_x1[:], x_fin[:])
            ps_gg = pagg.tile([128, 512], F32, tag="agg")
            for b in range(BPC):
                bs = b * 128
                ps_xn = psm.tile([128, 128], F32, tag="sm")
                nc.tensor.matmul(ps_xn[:], lhsT=x_fin[:, bs : bs + 128],
                                 rhs=ident_sb[:], is_transpose=True,
                                 start=True, stop=True)
                xn = wp.tile([128, 128], F32, tag="xn")
                nc.vector.tensor_copy(xn[:], ps_xn[:])
                Bt = wp.tile([128, 512], F32, tag="Bt")
                nc.vector.tensor_tensor(
                    out=Bt[:],
                    in0=bv_sb[:, b : b + 1].to_broadcast([128, 512]),
                    in1=iota512_sb[:],
                    op=mybir.AluOpType.is_equal)
                nc.tensor.matmul(ps_gg[:], lhsT=xn[:], rhs=Bt[:],
                                 start=(b == 0), stop=(b == BPC - 1))
            ggS = wp.tile([128, 512], F32, tag="ggS")
            nc.vector.tensor_copy(ggS[:], ps_gg[:])
            if debug:
                nc.sync.dma_start(dbg_gg[:], ggS[:])
            nc.gpsimd.dma_start(cc_in[:], ggS[:])
            if sim_mode:
                nc.sync.dma_start(cc_out.opt()[:, :], cc_in.opt()[:, :])
            else:
                nc.gpsimd.collective_compute(
                    "AllReduce", mybir.AluOpType.add,
                    replica_groups=[list(range(NCORES))],
                    ins=[cc_in.opt()], outs=[cc_out.opt()],
                )
            gsum = wp.tile([128, 512], F32, tag="gsum")
            nc.gpsimd.dma_start(gsum[:], cc_out[:])
            gmean = wp.tile([128, 512], F32, tag="gmean")
            nc.vector.tensor_tensor(out=gmean[:], in0=gsum[:], in1=invc_sb[:],
                                    op=mybir.AluOpType.mult)
            ps_y = pmid.tile([128, 512], F32, tag="ps_mn")
            nc.tensor.matmul(ps_y[:],
                             lhsT=w_sb[:, LAYERS * 640 : LAYERS * 640 + 128],
                             rhs=gmean[:], start=True, stop=True)
            y1 = wp.tile([128, 512], F32, tag="y1")
            nc.scalar.activation(y1[:], ps_y[:],
                                 mybir.ActivationFunctionType.Silu,
                                 bias=b_sb[:, 3 * LAYERS : 3 * LAYERS + 1])
            ps_p = psm.tile([1, 512], F32, tag="sm")
            nc.tensor.matmul(
                ps_p[:],
                lhsT=w_sb[:, LAYERS * 640 + 128 : LAYERS * 640 + 129],
                rhs=y1[:], start=True, stop=True)
            predS = wp.tile([1, 512], F32, tag="predS")
            nc.vector.tensor_tensor(
                out=predS[:], in0=ps_p[:],
                in1=misc_sb[0:1, RB2_OFF : RB2_OFF + 1].to_broadcast([1, 512]),
                op=mybir.AluOpType.add)
            nc.sync.dma_start(pred_t[:], predS[:])

    _split_excess_waits(nc)
    return nc


_prog_cache = {}


def kernel(**inputs) -> np.ndarray:
    per_core = _preprocess(
        inputs["z"], inputs["edge_index"], inputs["edge_attr"], inputs["batch"]
    )
    shared = _prep_weights(
        inputs["emb"], inputs["ew1"], inputs["eb1"], inputs["ew2"], inputs["eb2"],
        inputs["nw"], inputs["nb"], inputs["rw1"], inputs["rb1"], inputs["rw2"],
        inputs["rb2"], inputs["batch"],
    )
    if "prog" not in _prog_cache:
        _prog_cache["prog"] = _build_program()
    nc = _prog_cache["prog"]
    in_maps = [{**pc, **shared} for pc in per_core]
    res = run_bass_kernel_spmd(nc, in_maps, core_ids=list(range(NCORES)))
    return np.asarray(res.results[0]["pred"]).reshape(G).astype(np.float32)


# revision 29
# speedup vs baseline: 1.4790x; 1.0240x over previous
"""Trainium2 Bass kernel for the AdsorptionGNN message-passing network.

Strategy (8 NeuronCores, SPMD):
  - Permute nodes into 8*49 blocks of <=128 nodes with <=2048 in-edges each
    (greedy bin packing).  Core k owns 49 blocks (6250 nodes); edges are
    sharded by dst block, so the scatter-add is block-local in PSUM.
  - Per layer, each core computes u = x @ W1a and v = x @ W1b for its own
    nodes; u is AllGather-ed so every core can gather u[src] per edge
    (the only per-edge memory traffic).  The dst contribution uses the
    block-local one-hot matrix P (built on-chip with is_equal) so no
    x[dst] gather is needed:  h^T = u[src]^T + v_b^T @ P^T + W1c^T @ ea^T.
  - msg = silu(h @ ew2 + b2) ;  agg = P^T-matmul accumulation in PSUM;
    x' = silu([x|agg] @ nw + nb)  computed block by block.
  - Graph pooling: per-core partial sums via one-hot batch matmul,
    AllReduce over the 8 cores, then the tiny readout MLP on every core.
"""

import numpy as np

import concourse.bass as bass
import concourse.tile as tile
from concourse import mybir
from concourse.bass_utils import run_bass_kernel_spmd
from concourse.masks import make_identity

# -------------------- problem constants (hardcoded) --------------------
N_NODES = 50000
N_EDGES = 800000
HID = 128
ED = 4
LAYERS = 4
G = 512
NCORES = 8

BPC = 49              # blocks per core
CAP = 2048            # edge capacity per block
SCW = 512             # superchunk width (edges)
SC_PER_B = CAP // SCW # 4
CH_PER_SC = SCW // 128
NCH = BPC * CAP // 128          # 784 chunks of 128 edges per core
NSC = BPC * SC_PER_B            # 196 superchunks per core
SLOTS = BPC * 128               # 6272 node slots per core
EPC = BPC * CAP                 # 100352 padded edges per core

F32 = mybir.dt.float32
I32 = mybir.dt.int32

# misc row offsets (single-partition constants)
IOTA512_OFF = 0
INVC_OFF = 512
RB2_OFF = 1024
MISC_W = 1025


def _pack_nodes(deg):
    """Greedy bin packing: 392 bins, <=128 nodes and <=2048 edges per bin,
    exactly 6250 nodes per core.  Returns (bin_id[n], pos_in_bin[n])."""
    import heapq

    NB = NCORES * BPC
    order = np.argsort(-deg, kind="stable")
    bin_edges = np.zeros(NB, np.int64)
    bin_nodes = np.zeros(NB, np.int64)
    core_nodes = np.zeros(NCORES, np.int64)
    bin_id = np.full(N_NODES, -1, np.int64)
    pos = np.full(N_NODES, -1, np.int64)
    heap = [(-CAP, b) for b in range(NB)]
    heapq.heapify(heap)
    for n in order:
        dn = int(deg[n])
        tmp = []
        placed = False
        while heap:
            negrem, b = heapq.heappop(heap)
            core = b // BPC
            if (
                bin_edges[b] + dn <= CAP
                and bin_nodes[b] < 128
                and core_nodes[core] < N_NODES // NCORES
            ):
                bin_id[n] = b
                pos[n] = bin_nodes[b]
                bin_edges[b] += dn
                bin_nodes[b] += 1
                core_nodes[core] += 1
                heapq.heappush(heap, (-(CAP - bin_edges[b]), b))
                placed = True
                break
            tmp.append((negrem, b))
        for t in tmp:
            heapq.heappush(heap, t)
        if not placed:
            raise RuntimeError("node bin packing failed")
    return bin_id, pos


def _preprocess(z, edge_index, edge_attr, batch):
    """Host-side sharding.  Returns a list of per-core input dicts plus the
    shared-weight entries to be merged in."""
    z = np.asarray(z).astype(np.int64)
    src = np.asarray(edge_index[0]).astype(np.int64)
    dst = np.asarray(edge_index[1]).astype(np.int64)
    ea = np.asarray(edge_attr).astype(np.float32)
    batch = np.asarray(batch).astype(np.int64)

    deg = np.bincount(dst, minlength=N_NODES)
    bin_id, pos = _pack_nodes(deg)

    # global u-table row for each node: core*SLOTS + block*128 + pos
    core_of = bin_id // BPC
    block_of = bin_id % BPC
    urow = core_of * SLOTS + block_of * 128 + pos

    # order edges by dst bin (stable)
    e_bin = bin_id[dst]
    e_order = np.argsort(e_bin, kind="stable")
    sorted_bin = e_bin[e_order]
    # rank of each edge within its bin
    bin_counts = np.bincount(sorted_bin, minlength=NCORES * BPC)
    assert bin_counts.max() <= CAP
    starts = np.zeros(NCORES * BPC + 1, np.int64)
    np.cumsum(bin_counts, out=starts[1:])
    q = np.arange(N_EDGES) - starts[sorted_bin]
    e_core = sorted_bin // BPC
    e_block = sorted_bin % BPC
    j = e_block * CAP + q  # slot within core's padded edge list

    per_core = []
    for c in range(NCORES):
        m = e_core == c
        js = j[m]
        es = e_order[m]
        src_cols = np.zeros((128, NCH), np.int32)
        dst_cols = np.full((128, NCH), -1.0, np.float32)
        ea_pack = np.zeros((4, EPC), np.float32)
        pcol = (js % 128).astype(np.int64)
        ccol = (js // 128).astype(np.int64)
        src_cols[pcol, ccol] = urow[src[es]].astype(np.int32)
        dst_cols[pcol, ccol] = pos[dst[es]].astype(np.float32)
        for d in range(ED):
            ea_pack[d, js] = ea[es, d]
        dstr = np.full((1, EPC), -1.0, np.float32)
        dstr[0, js] = pos[dst[es]].astype(np.float32)

        z_rows = np.zeros((1, SLOTS), np.float32)
        z_cols = np.zeros((128, BPC), np.int32)
        bv_cols = np.full((128, BPC), -1.0, np.float32)
        nm = core_of == c
        nidx = np.nonzero(nm)[0]
        z_cols[pos[nidx], block_of[nidx]] = z[nidx].astype(np.int32)
        z_rows[0, block_of[nidx] * 128 + pos[nidx]] = z[nidx].astype(np.float32)
        bv_cols[pos[nidx], block_of[nidx]] = batch[nidx].astype(np.float32)
        per_core.append(
            {
                "src_cols": src_cols,
                "dst_cols": dst_cols,
                "ea_pack": ea_pack,
                "dstr": dstr,
                "z_rows": z_rows,
                "bv_cols": bv_cols,
            }
        )
    return per_core


def _prep_weights(emb, ew1, eb1, ew2, eb2, nw, nb, rw1, rb1, rw2, rb2, batch):
    emb = np.asarray(emb, np.float32)
    ew1 = np.asarray(ew1, np.float32)
    ew2 = np.asarray(ew2, np.float32)
    nw = np.asarray(nw, np.float32)
    rw1 = np.asarray(rw1, np.float32)
    rw2 = np.asarray(rw2, np.float32)

    wpack = np.zeros((128, LAYERS * 640 + 129), np.float32)
    for l in range(LAYERS):
        b = l * 640
        wpack[:, b : b + 128] = ew1[l][:128]
        wpack[:, b + 128 : b + 256] = ew1[l][128:256]
        wpack[:, b + 256 : b + 384] = ew2[l]
        wpack[:, b + 384 : b + 512] = nw[l][:128]
        wpack[:, b + 512 : b + 640] = nw[l][128:]
    wpack[:, LAYERS * 640 : LAYERS * 640 + 128] = rw1
    wpack[:, LAYERS * 640 + 128 : LAYERS * 640 + 129] = rw2

    w1c = np.zeros((4, 128 * LAYERS), np.float32)
    for l in range(LAYERS):
        w1c[:, 128 * l : 128 * (l + 1)] = ew1[l][256:260]

    bpack = np.zeros((128, 3 * LAYERS + 1), np.float32)
    for l in range(LAYERS):
        bpack[:, 3 * l] = np.asarray(eb1[l], np.float32)
        bpack[:, 3 * l + 1] = np.asarray(eb2[l], np.float32)
        bpack[:, 3 * l + 2] = np.asarray(nb[l], np.float32)
    bpack[:, 3 * LAYERS] = np.asarray(rb1, np.float32)

    counts = np.bincount(np.asarray(batch, np.int64), minlength=G).astype(np.float32)
    misc = np.zeros((1, MISC_W), np.float32)
    misc[0, IOTA512_OFF : IOTA512_OFF + 512] = np.arange(512, dtype=np.float32)
    misc[0, INVC_OFF : INVC_OFF + 512] = 1.0 / np.maximum(counts, 1.0)
    misc[0, RB2_OFF] = float(np.asarray(rb2).reshape(-1)[0])

    iota_mat = np.tile(np.arange(128, dtype=np.float32)[None, :], (128, 1))
    iota_col = np.arange(128, dtype=np.float32).reshape(128, 1)
    iota512_mat = np.tile(np.arange(512, dtype=np.float32)[None, :], (128, 1))
    invc_mat = np.tile(misc[0:1, INVC_OFF : INVC_OFF + 512], (128, 1))

    return {
        "wpack": wpack,
        "w1c": w1c,
        "bpack": bpack,
        "misc": misc,
        "iota_mat": iota_mat,
        "iota_col": iota_col,
        "iota512_mat": iota512_mat,
        "invc_mat": invc_mat,
        "emb_t": emb,
    }


def _split_excess_waits(nc, max_waits=1):
    """This walrus build rejects instructions with >1 embedded sync wait.
    Hoist excess waits onto same-engine NoOps inserted just before."""
    cnt = 0
    for f in nc.m.functions:
        for bb in f.blocks:
            new_list = []
            for ins in bb.instructions:
                si = ins.sync_info
                waits = list(si.on_wait) if si and si.on_wait else []
                if len(waits) > max_waits:
                    extra = waits[:-max_waits]
                    si.on_wait = waits[-max_waits:]
                    while extra:
                        chunk, extra = extra[:max_waits], extra[max_waits:]
                        cnt += 1
                        nop = mybir.InstNoOp(
                            name=f"I-waitsplit-{cnt}", engine=ins.engine, ins=[], outs=[]
                        )
                        nop.sync_info = mybir.SyncInfo(on_wait=chunk, on_update=[])
                        new_list.append(nop)
                new_list.append(ins)
            bb.instructions[:] = new_list
    return cnt


def _build_program(n_layers=LAYERS, debug=False, sim_mode=False):
    nc = bass.Bass("TRN2", target_bir_lowering=False, num_swdge_queues=2)

    ei = lambda name, shape, dt=F32: nc.dram_tensor(name, shape, dt, kind="ExternalInput")
    src_t = ei("src_cols", [128, NCH], I32)
    dstc_t = ei("dst_cols", [128, NCH])
    ea_t = ei("ea_pack", [4, EPC])
    dstr_t = ei("dstr", [1, EPC])
    zrow_t = ei("z_rows", [1, SLOTS])
    bv_t = ei("bv_cols", [128, BPC])
    w_t = ei("wpack", [128, LAYERS * 640 + 129])
    w1c_t = ei("w1c", [4, 128 * LAYERS])
    b_t = ei("bpack", [128, 3 * LAYERS + 1])
    misc_t = ei("misc", [1, MISC_W])
    iota_t = ei("iota_mat", [128, 128])
    iotac_t = ei("iota_col", [128, 1])
    iota512_t = ei("iota512_mat", [128, 512])
    invc_t = ei("invc_mat", [128, 512])
    emb_t = ei("emb_t", [101, 128])
    pred_t = nc.dram_tensor("pred", [1, G], F32, kind="ExternalOutput")
    if debug:
        dbg_x0 = nc.dram_tensor("dbg_x0", [128, SLOTS], F32, kind="ExternalOutput")
        dbg_u = nc.dram_tensor("dbg_u", [SLOTS, 128], F32, kind="ExternalOutput")
        dbg_h = nc.dram_tensor("dbg_h", [128, 512], F32, kind="ExternalOutput")
        dbg_mN = nc.dram_tensor("dbg_mN", [128, 512], F32, kind="ExternalOutput")
        dbg_agg = nc.dram_tensor("dbg_agg", [128, 128], F32, kind="ExternalOutput")
        dbg_x1 = nc.dram_tensor("dbg_x1", [128, SLOTS], F32, kind="ExternalOutput")
        dbg_gg = nc.dram_tensor("dbg_gg", [128, 512], F32, kind="ExternalOutput")
        dbg_us = nc.dram_tensor("dbg_us", [128, 512], F32, kind="ExternalOutput")

    with tile.TileContext(nc) as tc:
        with tc.tile_pool(name="persist", bufs=1) as pp, \
             tc.tile_pool(name="work", bufs=3) as wp, \
             tc.tile_pool(name="gat", bufs=(12 if debug else 20)) as gp, \
             tc.tile_pool(name="ptp", bufs=4) as ptp, \
             tc.tile_pool(name="ps_big", bufs=2, space="PSUM") as pbig, \
             tc.tile_pool(name="ps_mid", bufs=2, space="PSUM") as pmid, \
             tc.tile_pool(name="ps_sm", bufs=2, space="PSUM") as psm, \
             tc.tile_pool(name="ps_agg", bufs=2, space="PSUM") as pagg, \
             tc.tile_pool(name="dram", bufs=1, space="DRAM") as dp:

            # ---------------- persistent SBUF state ----------------
            src_sb = pp.tile([128, NCH], I32)
            dstc_sb = pp.tile([128, NCH], F32)
            bv_sb = pp.tile([128, BPC], F32)
            w_sb = pp.tile([128, LAYERS * 640 + 129], F32)
            w1c_sb = pp.tile([4, 128 * LAYERS], F32)
            b_sb = pp.tile([128, 3 * LAYERS + 1], F32)
            misc_sb = pp.tile([1, MISC_W], F32)
            iota_sb = pp.tile([128, 128], F32)
            iotac_sb = pp.tile([128, 1], F32)
            iota512_sb = pp.tile([128, 512], F32)
            invc_sb = pp.tile([128, 512], F32)
            ident_sb = pp.tile([128, 128], F32)
            xA = pp.tile([128, SLOTS], F32)
            xB = pp.tile([128, SLOTS], F32)
            v_sb = pp.tile([128, SLOTS], F32)

            for t, d in [
                (src_sb, src_t), (dstc_sb, dstc_t),
                (bv_sb, bv_t), (w_sb, w_t), (w1c_sb, w1c_t), (b_sb, b_t),
                (misc_sb, misc_t), (iota_sb, iota_t), (iotac_sb, iotac_t),
                (iota512_sb, iota512_t),
                (invc_sb, invc_t),
            ]:
                nc.sync.dma_start(t[:], d[:])
            make_identity(nc, ident_sb[:])

            u_own = dp.tile([SLOTS, 128], F32)
            u_fulls = [
                dp.tile([NCORES * SLOTS, 128], F32, addr_space="Shared",
                        name=f"u_full_l{i}", tag=f"u_full_l{i}")
                for i in range(n_layers)
            ]
            cc_in = dp.tile([128, 512], F32)
            cc_out = dp.tile([128, 512], F32, addr_space="Shared")

            WL = lambda l, k: w_sb[:, l * 640 + k * 128 : l * 640 + (k + 1) * 128]

            # ---------------- x0 = emb[z] (feature-major in SBUF) ----------------
            emb_sb = pp.tile([101, 128], F32)
            nc.sync.dma_start(emb_sb[:], emb_t[:])
            for b in range(BPC):
                bs = b * 128
                zbc = wp.tile([101, 128], F32, tag="zbc")
                nc.sync.dma_start(
                    zbc[:],
                    zrow_t[0:1, bs : bs + 128].to_broadcast([101, 128]))
                zoh = wp.tile([101, 128], F32, tag="zoh")
                nc.vector.tensor_tensor(
                    out=zoh[:],
                    in0=iotac_sb[0:101, 0:1].to_broadcast([101, 128]),
                    in1=zbc[:],
                    op=mybir.AluOpType.is_equal)
                ps = psm.tile([128, 128], F32, tag="sm")
                nc.tensor.matmul(ps[:], lhsT=emb_sb[:], rhs=zoh[:],
                                 start=True, stop=True)
                nc.vector.tensor_copy(xA[:, bs : bs + 128], ps[:])
            if debug:
                nc.sync.dma_start(dbg_x0[:], xA[:])

            # ---------------- layers ----------------
            def emit_uv(lw, x_src, b):
                """u/v for layer lw, block b, from feature-major x_src."""
                bs = b * 128
                ps_uv = psm.tile([128, 256], F32, tag="sm", name="ps_uv")
                nc.tensor.matmul(ps_uv[:, 0:128], lhsT=x_src[:, bs : bs + 128],
                                 rhs=WL(lw, 0), start=True, stop=True)
                nc.tensor.matmul(ps_uv[:, 128:256], lhsT=x_src[:, bs : bs + 128],
                                 rhs=WL(lw, 1), start=True, stop=True)
                ust = wp.tile([128, 128], F32, tag="ust", name="ust")
                nc.scalar.copy(ust[:], ps_uv[:, 0:128])
                nc.vector.tensor_copy(v_sb[:, bs : bs + 128], ps_uv[:, 128:256])
                nc.sync.dma_start(u_own[bs : bs + 128, :], ust[:])
                if debug and lw == 0:
                    nc.sync.dma_start(dbg_u[bs : bs + 128, :], ust[:])

            ps_gg = pagg.tile([128, 512], F32, tag="agg")
            for l in range(n_layers):
                x_cur = xA if l % 2 == 0 else xB
                x_nxt = xB if l % 2 == 0 else xA

                if l == 0:
                    # phase 1 standalone (later layers interleave it into
                    # the previous layer's block loop)
                    for b in range(BPC):
                        emit_uv(0, x_cur, b)

                # --- AllGather u across the 8 cores ---
                u_full = u_fulls[l]
                if sim_mode:
                    nc.sync.dma_start(u_full.opt()[0:SLOTS, :], u_own.opt()[:, :])
                else:
                    nc.gpsimd.collective_compute(
                        "AllGather", mybir.AluOpType.bypass,
                        replica_groups=[list(range(NCORES))],
                        ins=[u_own.opt()], outs=[u_full.opt()],
                    )

                # --- phase 2: edges ---
                for b in range(BPC):
                    bs = b * 128
                    ps_agg = pagg.tile([128, 128], F32, tag="agg")
                    for sc in range(SC_PER_B):
                        s = b * SC_PER_B + sc
                        ps_h = pbig.tile([128, 512], F32, tag="ps_h")
                        ptiles = []
                        gtiles = []
                        for ccn in range(CH_PER_SC):
                            c = s * CH_PER_SC + ccn
                            g = gp.tile([128, 128], F32, tag="g")
                            gi = nc.gpsimd.indirect_dma_start(
                                out=g[:], out_offset=None, in_=u_full.opt(),
                                in_offset=bass.IndirectOffsetOnAxis(
                                    ap=src_sb[:, c : c + 1], axis=0),
                            )
                            if c % 2 == 1:
                                # alternate the two SWDGE queues so both Q7
                                # descriptor-gen cores work in parallel
                                gi.ins.queue = "qPoolDynamic1"
                            gtiles.append(g)
                        # one-hot P for all 4 chunks in one op:
                        # in0[p, ccn*128+n] = dstc[p, c0+ccn]; in1 repeats iota
                        c0 = s * CH_PER_SC
                        pt4 = ptp.tile([128, 512], F32, tag="pt")
                        nc.vector.tensor_tensor(
                            out=pt4[:],
                            in0=dstc_sb[:, c0 : c0 + 4].rearrange(
                                "p (c o) -> p c o", o=1).to_broadcast([128, 4, 128]),
                            in1=iota_sb[:].rearrange(
                                "p (o n) -> p o n", o=1).to_broadcast([128, 4, 128]),
                            op=mybir.AluOpType.is_equal)
                        ptiles = [pt4[:, ccn * 128 : (ccn + 1) * 128]
                                  for ccn in range(CH_PER_SC)]
                        # P^T built directly: broadcast dst row across
                        # partitions during DMA, then one is_equal vs iota col
                        dstr_bc = wp.tile([128, 512], F32, tag="dstr_bc")
                        nc.sync.dma_start(
                            dstr_bc[:],
                            dstr_t[0:1, s * SCW : s * SCW + SCW].to_broadcast(
                                [128, 512]))
                        pT = wp.tile([128, 512], F32, tag="pT")
                        nc.vector.tensor_tensor(
                            out=pT[:],
                            in0=iotac_sb[:, 0:1].to_broadcast([128, 512]),
                            in1=dstr_bc[:],
                            op=mybir.AluOpType.is_equal)
                        # h^T += v_b^T P^T  +  W1c^T ea^T
                        ea_sl = wp.tile([4, 512], F32, tag="ea_sl")
                        nc.sync.dma_start(ea_sl[:], ea_t[:, s * SCW : s * SCW + SCW])
                        nc.tensor.matmul(ps_h[:], lhsT=v_sb[:, bs : bs + 128],
                                         rhs=pT[:], start=True, stop=False)
                        nc.tensor.matmul(
                            ps_h[:], lhsT=w1c_sb[:, 128 * l : 128 * (l + 1)],
                            rhs=ea_sl[:], start=False, stop=False)
                        for ccn in range(CH_PER_SC):
                            nc.tensor.matmul(
                                ps_h[:, ccn * 128 : (ccn + 1) * 128],
                                lhsT=gtiles[ccn][:],
                                rhs=ident_sb[:], is_transpose=True,
                                start=False, stop=(ccn == CH_PER_SC - 1))
                        hT = wp.tile([128, 512], F32, tag="hT")
                        nc.scalar.activation(hT[:], ps_h[:],
                                             mybir.ActivationFunctionType.Silu,
                                             bias=b_sb[:, 3 * l : 3 * l + 1])
                        if debug and l == 0 and s == 0:
                            nc.sync.dma_start(dbg_h[:], hT[:])
                            dbg_us_t = wp.tile([128, 512], F32, tag="dbg_us_t")
                            nc.vector.tensor_copy(dbg_us_t[:], ps_h[:])
                            nc.sync.dma_start(dbg_us[:], dbg_us_t[:])
                        ps_m = pmid.tile([128, 512], F32, tag="ps_mn")
                        nc.tensor.matmul(ps_m[:], lhsT=WL(l, 2), rhs=hT[:],
                                         start=True, stop=True)
                        mT = wp.tile([128, 512], F32, tag="mT")
                        nc.scalar.activation(mT[:], ps_m[:],
                                             mybir.ActivationFunctionType.Silu,
                                             bias=b_sb[:, 3 * l + 1 : 3 * l + 2])
                        # transpose msg to edge-major via PE
                        ps_n = pmid.tile([128, 512], F32, tag="ps_mn")
                        for ccn in range(CH_PER_SC):
                            nc.tensor.matmul(
                                ps_n[:, ccn * 128 : (ccn + 1) * 128],
                                lhsT=mT[:, ccn * 128 : (ccn + 1) * 128],
                                rhs=ident_sb[:], is_transpose=True,
                                start=True, stop=True)
                        mN = wp.tile([128, 512], F32, tag="mN")
                        nc.vector.tensor_copy(mN[:], ps_n[:])
                        if debug and l == 0 and s == 0:
                            nc.sync.dma_start(dbg_mN[:], mN[:])
                        for ccn in range(CH_PER_SC):
                            nc.tensor.matmul(
                                ps_agg[:], lhsT=ptiles[ccn],
                                rhs=mN[:, ccn * 128 : (ccn + 1) * 128],
                                start=(sc == 0 and ccn == 0),
                                stop=(sc == SC_PER_B - 1 and ccn == CH_PER_SC - 1))
                    # --- node update for block b ---
                    aggS = wp.tile([128, 128], F32, tag="aggS")
                    nc.vector.tensor_copy(aggS[:], ps_agg[:])
                    if debug and l == 0 and b == 0:
                        nc.sync.dma_start(dbg_agg[:], aggS[:])
                    ps_aT = psm.tile([128, 128], F32, tag="sm")
                    nc.tensor.matmul(ps_aT[:], lhsT=aggS[:], rhs=ident_sb[:],
                                     is_transpose=True, start=True, stop=True)
                    aggT = wp.tile([128, 128], F32, tag="aggT")
                    nc.vector.tensor_copy(aggT[:], ps_aT[:])
                    ps_x = psm.tile([128, 128], F32, tag="sm")
                    nc.tensor.matmul(ps_x[:], lhsT=WL(l, 3),
                                     rhs=x_cur[:, bs : bs + 128],
                                     start=True, stop=False)
                    nc.tensor.matmul(ps_x[:], lhsT=WL(l, 4), rhs=aggT[:],
                                     start=False, stop=True)
                    nc.scalar.activation(x_nxt[:, bs : bs + 128], ps_x[:],
                                         mybir.ActivationFunctionType.Silu,
                                         bias=b_sb[:, 3 * l + 2 : 3 * l + 3])
                    if l + 1 < n_layers:
                        # u/v for the next layer, hidden in this layer
                        emit_uv(l + 1, x_nxt, b)
                    else:
                        # graph pooling contribution, hidden in last layer
                        ps_xn = psm.tile([128, 128], F32, tag="sm")
                        nc.tensor.matmul(ps_xn[:], lhsT=x_nxt[:, bs : bs + 128],
                                         rhs=ident_sb[:], is_transpose=True,
                                         start=True, stop=True)
                        xn = wp.tile([128, 128], F32, tag="xn")
                        nc.vector.tensor_copy(xn[:], ps_xn[:])
                        Bt = wp.tile([128, 512], F32, tag="Bt")
                        nc.vector.tensor_tensor(
                            out=Bt[:],
                            in0=bv_sb[:, b : b + 1].to_broadcast([128, 512]),
                            in1=iota512_sb[:],
                            op=mybir.AluOpType.is_equal)
                        nc.tensor.matmul(ps_gg[:], lhsT=xn[:], rhs=Bt[:],
                                         start=(b == 0), stop=(b == BPC - 1))

            # ---------------- readout ----------------
            x_fin = xA if n_layers % 2 == 0 else xB
            if debug:
                nc.sync.dma_start(dbg_x1[:], x_fin[:])
            ggS = wp.tile([128, 512], F32, tag="ggS")
            nc.vector.tensor_copy(ggS[:], ps_gg[:])
            if debug:
                nc.sync.dma_start(dbg_gg[:], ggS[:])
            nc.gpsimd.dma_start(cc_in[:], ggS[:])
            if sim_mode:
                nc.sync.dma_start(cc_out.opt()[:, :], cc_in.opt()[:, :])
            else:
                nc.gpsimd.collective_compute(
                    "AllReduce", mybir.AluOpType.add,
                    replica_groups=[list(range(NCORES))],
                    ins=[cc_in.opt()], outs=[cc_out.opt()],
                )
            gsum = wp.tile([128, 512], F32, tag="gsum")
            nc.gpsimd.dma_start(gsum[:], cc_out[:])
            gmean = wp.tile([128, 512], F32, tag="gmean")
            nc.vector.tensor_tensor(out=gmean[:], in0=gsum[:], in1=invc_sb[:],
                                    op=mybir.AluOpType.mult)
            ps_y = pmid.tile([128, 512], F32, tag="ps_mn")
            nc.tensor.matmul(ps_y[:],
                             lhsT=w_sb[:, LAYERS * 640 : LAYERS * 640 + 128],
                             rhs=gmean[:], start=True, stop=True)
            y1 = wp.tile([128, 512], F32, tag="y1")
            nc.scalar.activation(y1[:], ps_y[:],
                                 mybir.ActivationFunctionType.Silu,
                                 bias=b_sb[:, 3 * LAYERS : 3 * LAYERS + 1])
            ps_p = psm.tile([1, 512], F32, tag="sm")
            nc.tensor.matmul(
                ps_p[:],
                lhsT=w_sb[:, LAYERS * 640 + 128 : LAYERS * 640 + 129],
                rhs=y1[:], start=True, stop=True)
            predS = wp.tile([1, 512], F32, tag="predS")
            nc.vector.tensor_tensor(
                out=predS[:], in0=ps_p[:],
                in1=misc_sb[0:1, RB2_OFF : RB2_OFF + 1].to_broadcast([1, 512]),
                op=mybir.AluOpType.add)
            nc.sync.dma_start(pred_t[:], predS[:])

    _split_excess_waits(nc)
    return nc


_prog_cache = {}


def kernel(**inputs) -> np.ndarray:
    per_core = _preprocess(
        inputs["z"], inputs["edge_index"], inputs["edge_attr"], inputs["batch"]
    )
    shared = _prep_weights(
        inputs["emb"], inputs["ew1"], inputs["eb1"], inputs["ew2"], inputs["eb2"],
        inputs["nw"], inputs["nb"], inputs["rw1"], inputs["rb1"], inputs["rw2"],
        inputs["rb2"], inputs["batch"],
    )
    if "prog" not in _prog_cache:
        _prog_cache["prog"] = _build_program()
    nc = _prog_cache["prog"]
    in_maps = [{**pc, **shared} for pc in per_core]
    res = run_bass_kernel_spmd(nc, in_maps, core_ids=list(range(NCORES)))
    return np.asarray(res.results[0]["pred"]).reshape(G).astype(np.float32)
